# revision 29
# baseline (speedup 1.0000x reference)
"""Trainium2 Bass kernel for a 12-head attention module (B=4, S=1024, E=256, H=12,
per-head dim = E — the module quirk that makes per-head weight fusion possible).

Sharding: 8 cores = 4 batches x 2 head-groups (6 heads each).  Each core computes
its partial fc projection; the host sums the two partials per batch element.

Algebraic fusion (host precomputes, in float64):
  wa_h = scale * Wk_h @ Wq_h^T  (so uT = wa^T @ xT and scoresT = uT^T-contracted
  xT: the q/k projections collapse into one matmul chain; qT/kT never exist).
  wc_h = Wv_h @ Wfc_h  (the fc layer disappears: w_h = x @ wc_h,
  out = sum_h softmax(scores_h) @ w_h).  bv/bfc become an exact host-side
  constant row; nonzero bq reduces to a per-key bias on the exp.

Dtype strategy (empirically validated, rel_l2 ~6e-3 vs 2e-2 budget):
  * scores path in float8e4 (e4m3) with DoubleRow perf mode: each matmul
    contracts 2x128 rows at 0.5 PE-cycles per output column — 4x fewer PE
    cycles than fp32r.  wa is pre-scaled by 2^11 so fp8 sees well-scaled
    values; the exp activation applies scale=2^-11 to undo it.
  * w = x @ wc also runs as fp8 DoubleRow via an exact hi/lo split done on
    the host: w*2^8 = x_hi@C_hi + (16*x_lo)@(C_hi/16) + (x_hi/16)@(16*C_lo),
    three DoubleRow matmuls (1.5 cycles/col) vs two bf16 ones (2 cycles/col),
    with bf16-level accuracy since the dropped lo*lo term is O(2^-8).
  * probs and w live in fp32r (exact in PSUM->SBUF); the ctx matmul runs
    fp32r at full speed (free dim 258 >= 256).  probs in fp8 or the w chain
    in pure fp8 FAILS the error budget (their quantization error lands on
    the output without averaging down).

The 2^8 scale on w cancels for free: the ones column appended to w (for the
softmax denominators in PSUM column 256) is set to 2^8, so the reciprocal of
that column is 2^-8/denom and one fused scalar_tensor_tensor per output block
(acc = psum*rec + acc, on DVE) both normalizes and undoes the scale.

Engine roles (GPSIMD/Pool cannot touch PSUM on TRN2):
  PE   — all matmuls;  Act — exp + one uT PSUM->SBUF fp8 copy per head;
  DVE  — the other uT copy, reciprocals, fused normalize+acc;
  Pool — causal-mask multiplies on probs (SBUF), ones-column memsets, and
         SWDGE issue of the w PSUM->SBUF DMAs (the transfer itself runs on
         the shared DMA engines, off every compute engine's timeline).

The per-head program is software-pipelined: head h's ctx matmuls are
interleaved into head h+1's uT/w/scores phases so the PE never waits on the
exp engine and PSUM tile-pool rotation stalls are covered.
"""

import numpy as np
import ml_dtypes

import concourse.mybir as mybir
import concourse.tile as tile
from concourse import bacc
from concourse.bass_utils import run_bass_kernel_spmd

# Problem constants
B, S, E, H = 4, 1024, 256, 12
P = 128
NCORES = 8
HPC = H // 2            # heads per core
EH = E * HPC            # 1536 = per-core head width
KS_E = E // P           # 2 contraction subtiles over E
ST = S // P             # 8 row-blocks of S
EWW = E + 2             # ww width incl. ones cols (even, for fp32r matmul)

KA = 11                 # wa pre-scale exponent (fp8 dynamic range centering)
KC = 8                  # wc pre-scale exponent; ones column = 2^KC
F8MAX = 240.0           # float8e4 (IEEE e4m3) max finite

F8 = mybir.dt.float8e4
BF = mybir.dt.bfloat16
F32 = mybir.dt.float32
F32R = mybir.dt.float32r
NF8 = ml_dtypes.float8_e4m3
NBF = ml_dtypes.bfloat16
DR = mybir.MatmulPerfMode.DoubleRow

LAST_RESULTS = None     # BassKernelResults of the most recent run (for harness)


def _mask_structure(attention_mask):
    """Analyze the additive mask into per-key-block spans and multiply blocks.

    Returns (struct, emuls) where emuls is [nuniq, P, P] float32 exp(mask^T)
    blocks for the non-trivial (mixed) 128x128 tiles.
    """
    m = np.asarray(attention_mask, dtype=np.float64).reshape(S, S)   # [q, k]
    em = np.exp(m).astype(np.float32)
    emT = np.ascontiguousarray(em.T)                                 # [k, q]

    uniq: dict[bytes, int] = {}
    blocks = {}
    for ki in range(ST):
        for qj in range(ST):
            blk = np.ascontiguousarray(emT[ki * P:(ki + 1) * P, qj * P:(qj + 1) * P])
            if not blk.any():
                blocks[(ki, qj)] = "skip"
            elif (blk == 1.0).all():
                blocks[(ki, qj)] = "one"
            else:
                blocks[(ki, qj)] = uniq.setdefault(blk.tobytes(), len(uniq))

    zkey = np.zeros((P, P), np.float32).tobytes()
    spans = []
    mixed = []
    for ki in range(ST):
        non = [qj for qj in range(ST) if blocks[(ki, qj)] != "skip"]
        if not non:
            spans.append(None)
            continue
        qa, qb = non[0] * P, (non[-1] + 1) * P
        spans.append((qa, qb))
        for qj in range(qa // P, qb // P):
            bl = blocks[(ki, qj)]
            if bl == "one":
                continue
            if bl == "skip":  # interior hole: multiply by zeros
                bl = uniq.setdefault(zkey, len(uniq))
            mixed.append((ki, qj, bl))

    offs, tot = [], 0
    for ki in range(ST):
        offs.append(tot)
        if spans[ki] is not None:
            tot += spans[ki][1] - spans[ki][0]

    covers = []
    for m_ in range(ST):
        ks = tuple(ki for ki in range(ST)
                   if spans[ki] is not None
                   and spans[ki][0] <= m_ * P and spans[ki][1] >= (m_ + 1) * P)
        assert ks, (
            "attention row-block with no unmasked keys is not supported "
            "(reference softmax of an all-masked row is uniform)")
        covers.append(ks)

    # exp groups: greedy consecutive-ki packing, total width <= 1024 (2 banks)
    groups = []
    curg, curw = [], 0
    for ki in range(ST):
        if spans[ki] is None:
            continue
        w = spans[ki][1] - spans[ki][0]
        if curg and curw + w > 1024:
            groups.append((tuple(curg), curw))
            curg, curw = [], 0
        curg.append(ki)
        curw += w
    if curg:
        groups.append((tuple(curg), curw))

    nuniq = max(len(uniq), 1)
    emuls = np.ones((nuniq, P, P), np.float32)
    for key, uid in uniq.items():
        emuls[uid] = np.frombuffer(key, np.float32).reshape(P, P)

    struct = (tuple(spans), tuple(covers), tuple(offs), tot, tuple(groups),
              tuple(mixed), nuniq)
    return struct, emuls


def _build(struct, has_qk_bias):
    spans, covers, offs, tot, groups, mixed, nuniq = struct
    Exp = mybir.ActivationFunctionType.Exp
    MULT = mybir.AluOpType.mult
    ADD = mybir.AluOpType.add
    mixed_by_ki = {}
    for ki, qj, uid in mixed:
        mixed_by_ki.setdefault(ki, []).append((qj, uid))

    nc = bacc.Bacc("TRN2")
    xT8_d = nc.dram_tensor("xT8", (E, S), F8, kind="ExternalInput")
    xlo_d = nc.dram_tensor("xlo", (E, S), F8, kind="ExternalInput")
    xhid_d = nc.dram_tensor("xhid", (E, S), F8, kind="ExternalInput")
    wa8_d = nc.dram_tensor("wa8", (E, EH), F8, kind="ExternalInput")
    chi_d = nc.dram_tensor("chi", (E, EH), F8, kind="ExternalInput")
    chid_d = nc.dram_tensor("chid", (E, EH), F8, kind="ExternalInput")
    clo_d = nc.dram_tensor("clo", (E, EH), F8, kind="ExternalInput")
    emul_d = nc.dram_tensor("emul", (nuniq, P, P), F32R, kind="ExternalInput")
    wones_d = nc.dram_tensor("wones", (P, ST * 2 * (EWW - E)), F32R,
                             kind="ExternalInput")
    xTbf_d = nc.dram_tensor("xTbf", (E, S), BF, kind="ExternalInput")
    wm2_d = nc.dram_tensor("wm2", (E, 2 * HPC), BF, kind="ExternalInput")
    y_d = nc.dram_tensor("y", (S, E), F32, kind="ExternalOutput")

    with tile.TileContext(nc) as tc, \
            tc.tile_pool(name="singles", bufs=1) as singles, \
            tc.tile_pool(name="heads", bufs=2) as heads, \
            tc.tile_pool(name="small", bufs=4) as small, \
            tc.tile_pool(name="psS", bufs=2, space="PSUM") as psS, \
            tc.tile_pool(name="psC", bufs=4, space="PSUM") as psC:

        # ---- resident tensors, DMA'd in first-use order ----
        wa8_sb = singles.tile([P, KS_E, EH], F8)
        xT8_sb = singles.tile([P, KS_E, S], F8)
        xlo_sb = singles.tile([P, KS_E, S], F8)
        xhid_sb = singles.tile([P, KS_E, S], F8)
        chi_sb = singles.tile([P, KS_E, EH], F8)
        chid_sb = singles.tile([P, KS_E, EH], F8)
        clo_sb = singles.tile([P, KS_E, EH], F8)
        emul_sb = singles.tile([P, nuniq, P], F32R)
        h0 = slice(0, E)
        pair0 = slice(0, 2 * E)
        rest = slice(2 * E, EH)
        rr = "(ko p) n -> p ko n"
        # first-use order: uT (wa8 head0 + xT8), then exp masks, then the
        # h0 w-pair operands, then everything else.  The leading tensors are
        # split so the very first matmuls' inputs land as early as possible.
        nc.sync.dma_start(wa8_sb[:, :, 0:P], wa8_d[:, 0:P].rearrange(rr, p=P))
        nc.scalar.dma_start(xT8_sb[:, :, 0:512],
                            xT8_d[:, 0:512].rearrange(rr, p=P))
        nc.sync.dma_start(wa8_sb[:, :, P:E], wa8_d[:, P:E].rearrange(rr, p=P))
        nc.sync.dma_start(xT8_sb[:, :, 512:S],
                          xT8_d[:, 512:S].rearrange(rr, p=P))
        nc.sync.dma_start(emul_sb, emul_d[:, :, :].rearrange("u p q -> p u q"))
        nc.sync.dma_start(chi_sb[:, :, pair0], chi_d[:, pair0].rearrange(rr, p=P))
        nc.sync.dma_start(xlo_sb, xlo_d[:, :].rearrange(rr, p=P))
        nc.sync.dma_start(xhid_sb, xhid_d[:, :].rearrange(rr, p=P))
        nc.sync.dma_start(chid_sb[:, :, pair0], chid_d[:, pair0].rearrange(rr, p=P))
        nc.sync.dma_start(clo_sb[:, :, pair0], clo_d[:, pair0].rearrange(rr, p=P))
        nc.sync.dma_start(wa8_sb[:, :, slice(E, EH)],
                          wa8_d[:, slice(E, EH)].rearrange(rr, p=P))
        nc.sync.dma_start(chi_sb[:, :, rest], chi_d[:, rest].rearrange(rr, p=P))
        nc.sync.dma_start(chid_sb[:, :, rest], chid_d[:, rest].rearrange(rr, p=P))
        nc.sync.dma_start(clo_sb[:, :, rest], clo_d[:, rest].rearrange(rr, p=P))
        xTbf_sb = wm2_sb = None
        if has_qk_bias:
            xTbf_sb = singles.tile([P, KS_E, S], BF)
            wm2_sb = singles.tile([P, KS_E, 2 * HPC], BF)
            nc.sync.dma_start(xTbf_sb, xTbf_d[:, :].rearrange(rr, p=P))
            nc.sync.dma_start(wm2_sb, wm2_d[:, :].rearrange(rr, p=P))
        acc_sb = singles.tile([P, ST, E], F32)

        DIV = mybir.AluOpType.divide

        # ---- estimate-driven ctx scheduler ----------------------------
        # ctx units (one per finished head x output block) are emitted from
        # a global FIFO into the gaps of later heads' phases.  Norms are
        # deferred separately so the PSUM-evacuation op can be routed to
        # whichever of Act/DVE is behind (tracked by emitted-cost counters).
        est = {"PE": 0.0, "Act": 0.0, "DVE": 0.0}
        ctx_fifo = []     # (ph, m, probs, ww) pending units
        open_norms = []   # (ph, m, psc) matmuls emitted, norm pending

        def emit_unit_mms():
            ph, m_, probs_t, ww_t = ctx_fifo.pop(0)
            psc = psC.tile([P, EWW], F32, tag="ctx", name="ps_c")
            ks_list = covers[m_]
            last = len(ks_list) - 1
            for idx, ki in enumerate(ks_list):
                qa, _ = spans[ki]
                c0 = offs[ki] + m_ * P - qa
                nc.tensor.matmul(
                    psc,
                    probs_t[:, c0:c0 + P],
                    ww_t[:, ki, 0:EWW],
                    start=(idx == 0), stop=(idx == last),
                )
            est["PE"] += len(ks_list) * 108
            open_norms.append((ph, m_, psc))

        def emit_norm():
            ph, m_, psc = open_norms.pop(0)
            rec = small.tile([P, 1], F32, tag="rec")
            nc.vector.reciprocal(rec, psc[:, E:E + 1])
            est["DVE"] += 62
            route = "act" if est["Act"] + 460 < est["DVE"] + 392 else "dve"
            if route == "act":
                # Act+Pool normalize route: tmp = psum*rec on Act,
                # acc += tmp on Pool (SBUF-only)
                if ph == 0:
                    nc.scalar.mul(acc_sb[:, m_, :], psc[:, 0:E], rec)
                else:
                    tmp = small.tile([P, E], F32, tag="ntmp", bufs=2)
                    nc.scalar.mul(tmp, psc[:, 0:E], rec)
                    nc.gpsimd.tensor_add(acc_sb[:, m_, :], acc_sb[:, m_, :],
                                         tmp)
                est["Act"] += 398
            elif ph == 0:
                nc.vector.tensor_scalar_mul(acc_sb[:, m_, :], psc[:, 0:E], rec)
                est["DVE"] += 392
            else:
                nc.vector.scalar_tensor_tensor(
                    acc_sb[:, m_, :], psc[:, 0:E], rec, acc_sb[:, m_, :],
                    MULT, ADD)
                est["DVE"] += 392
            if ph == HPC - 1:
                nc.sync.dma_start(y_d[m_ * P:(m_ + 1) * P, :], acc_sb[:, m_, :])

        def fill():
            # keep the psC pool from gating the PE stream
            while len(open_norms) >= 3:
                emit_norm()
            if ctx_fifo and len(open_norms) < 3:
                emit_unit_mms()
            # drain norms toward the laggard aux engine while it's behind PE
            while open_norms and min(est["Act"], est["DVE"]) + 400 < est["PE"]:
                emit_norm()

        ww_pair = None
        for h in range(HPC):
            uT8 = heads.tile([P, KS_E, S], F8, tag="uT")
            probs = heads.tile([P, tot], F32R, tag="probs", bufs=3)

            # ---- uT = wa^T @ xT, fp8 DoubleRow; both e-tiles share one
            #      2-bank PSUM -> one wide PSUM->SBUF fp8 copy each ----
            for jn in range(2):
                ps = psS.tile([P, 1024], F32, tag="scores", name="ps_u")
                for t in range(KS_E):
                    nc.tensor.matmul(
                        ps[:, t * 512:(t + 1) * 512],
                        wa8_sb[:, :, h * E + t * P: h * E + (t + 1) * P],
                        xT8_sb[:, :, jn * 512:(jn + 1) * 512],
                        start=True, stop=True, perf_mode=DR,
                    )
                    est["PE"] += 107
                    fill()
                dst = uT8[:, 0:KS_E, jn * 512:(jn + 1) * 512]
                src = ps.rearrange("p (t n) -> p t n", t=KS_E)
                if jn == 0:
                    nc.scalar.copy(dst, src)
                    est["Act"] += 1038
                else:
                    nc.vector.tensor_copy(dst, src)
                    est["DVE"] += 1192

            # ---- s2_h = x @ wm2_h: per-key exp bias (only if bq != 0) ----
            s2 = None
            if has_qk_bias:
                s2 = heads.tile([P, ST, 2], F32, tag="s2")
                for st in range(ST):
                    ps = psC.tile([P, EWW], F32, tag="ctx", name="ps_s2")[:, :2]
                    for ks in range(KS_E):
                        nc.tensor.matmul(
                            ps,
                            xTbf_sb[:, ks, st * P:(st + 1) * P],
                            wm2_sb[:, ks, 2 * h:2 * h + 2],
                            start=(ks == 0), stop=(ks == KS_E - 1),
                        )
                    nc.vector.tensor_copy(s2[:, st, :], ps)

            def new_ww_pair():
                nonlocal ww_pair
                ww_pair = heads.tile([P, ST, 2, EWW], F32R, tag="wwp")
                nc.sync.dma_start(
                    ww_pair[:, :, :, E:EWW],
                    wones_d[:, :].rearrange("p (s a n) -> p s a n",
                                            s=ST, a=2))

            def do_w_tile(sp):
                # w*2^KC = x@wc via 3 fp8 DoubleRow terms for the head PAIR;
                # one PSUM tile covers two row-blocks
                hcols = slice(h * E, (h + 2) * E)
                ps = psS.tile([P, 1024], F32, tag="scores", name="ps_w")
                for si in range(2):
                    st = 2 * sp + si
                    xcols = slice(st * P, (st + 1) * P)
                    out = ps[:, si * 512:(si + 1) * 512]
                    nc.tensor.matmul(out, xT8_sb[:, :, xcols],
                                     chi_sb[:, :, hcols],
                                     start=True, stop=False, perf_mode=DR)
                    nc.tensor.matmul(out, xlo_sb[:, :, xcols],
                                     chid_sb[:, :, hcols],
                                     start=False, stop=False, perf_mode=DR)
                    nc.tensor.matmul(out, xhid_sb[:, :, xcols],
                                     clo_sb[:, :, hcols],
                                     start=False, stop=True, perf_mode=DR)
                nc.vector.tensor_copy(
                    ww_pair[:, 2 * sp:2 * sp + 2, :, 0:E],
                    ps.rearrange("p (s a n) -> p s a n", s=2, a=2))
                est["PE"] += 6 * 107
                est["DVE"] += 1192

            def do_w():
                new_ww_pair()
                for sp in range(ST // 2):
                    do_w_tile(sp)
                    fill()

            def do_scores(after_group=None):
                # scoresT -> exp -> (mask multiplies on Pool) => probs
                for gi, (kis, gw) in enumerate(groups):
                    pss = psS.tile([P, 1024], F32, tag="scores", name="ps_s")
                    goff = offs[kis[0]]
                    # chunks split at 512-col PSUM bank boundaries; per-bank
                    # start/stop accumulation chains
                    items = {0: [], 1: []}
                    for ki in kis:
                        qa, qb = spans[ki]
                        o = offs[ki] - goff
                        pos = o
                        while pos < o + (qb - qa):
                            end = min(o + (qb - qa), (pos // 512 + 1) * 512)
                            items[pos // 512].append((ki, pos, end))
                            pos = end
                    for bank in (0, 1):
                        blist = items[bank]
                        for idx, (ki, c0, c1) in enumerate(blist):
                            qa, _ = spans[ki]
                            o = offs[ki] - goff
                            pos0 = qa + (c0 - o)
                            nc.tensor.matmul(
                                pss[:, c0:c1],
                                uT8[:, :, ki * P:(ki + 1) * P],
                                xT8_sb[:, :, pos0:pos0 + (c1 - c0)],
                                start=(idx == 0), stop=(idx == len(blist) - 1),
                                perf_mode=DR,
                            )
                    est["PE"] += int(gw * 0.5 * 0.4167)
                    if not has_qk_bias:
                        nc.scalar.activation(
                            probs[:, goff:goff + gw], pss[:, 0:gw], Exp,
                            scale=2.0 ** -KA)
                        est["Act"] += int(gw * 0.833) + 185
                    else:
                        for ki in kis:
                            qa, qb = spans[ki]
                            o = offs[ki] - goff
                            nc.scalar.activation(
                                probs[:, offs[ki]:offs[ki] + qb - qa],
                                pss[:, o:o + qb - qa], Exp,
                                scale=2.0 ** -KA, bias=s2[:, ki, 0:1])
                    for ki in kis:
                        qa, _ = spans[ki]
                        for qj, uid in mixed_by_ki.get(ki, ()):
                            sl = slice(offs[ki] + qj * P - qa,
                                       offs[ki] + (qj + 1) * P - qa)
                            nc.gpsimd.tensor_mul(probs[:, sl], probs[:, sl],
                                                 emul_sb[:, uid, :])
                    if after_group is not None:
                        after_group(gi)
                    fill()

            if h == 0:
                # no ctx fill exists yet: interleave the w-pair tiles into
                # the scores phase — they fill the PE gaps while the exp
                # engine chases, and the w operands' DMAs get time to land
                new_ww_pair()
                wq = list(range(ST // 2))

                def w_filler(gi):
                    if wq:
                        do_w_tile(wq.pop(0))
                do_scores(after_group=w_filler)
                while wq:
                    do_w_tile(wq.pop(0))
            else:
                if h % 2 == 0:
                    do_w()
                do_scores()
            ww = ww_pair[:, :, h % 2, :]
            # interleaved small/large order so the matmul-count per fill
            # matches each phase's stall profile (w phase gets big units)
            for m_ in (0, 7, 1, 6, 2, 5, 3, 4):
                ctx_fifo.append((h, m_, probs, ww))

        # tail: drain remaining ctx units
        while ctx_fifo or open_norms:
            while len(open_norms) >= 3 or (not ctx_fifo and open_norms):
                emit_norm()
            if ctx_fifo:
                emit_unit_mms()

    nc.compile()
    return nc


_nc_cache = {}


def _q8(v):
    return np.clip(np.asarray(v, np.float32), -F8MAX, F8MAX).astype(NF8)


def make_core_inputs(x, attention_mask, Wq, bq, Wk, bk, Wv, bv, Wfc, bfc):
    """Host-side prep shared by kernel() and test harnesses.

    Returns (struct, has_qk_bias, in_maps, ybias).
    """
    x = np.asarray(x, np.float32)
    Wq64 = np.asarray(Wq, np.float64)
    Wk64 = np.asarray(Wk, np.float64)
    Wv64 = np.asarray(Wv, np.float64)
    Wfc64 = np.asarray(Wfc, np.float64)
    bq64 = np.asarray(bq, np.float64)
    bv64 = np.asarray(bv, np.float64)
    bfc64 = np.asarray(bfc, np.float64)

    has_qk_bias = bool(bq64.any())
    struct, emuls = _mask_structure(attention_mask)

    scale = 1.0 / np.sqrt(np.float64(E))
    wa = np.empty((E, E * H), np.float32)
    wcs = np.empty((E, E * H), np.float32)   # wc * 2^KC
    wm2 = np.zeros((E, 2 * H), np.float32)
    for g in range(H):
        gs = slice(g * E, (g + 1) * E)
        wa[:, gs] = (scale * (Wk64[:, gs] @ Wq64[:, gs].T) * 2.0 ** KA
                     ).astype(np.float32)
        wcs[:, gs] = ((Wv64[:, gs] @ Wfc64[gs, :]) * 2.0 ** KC
                      ).astype(np.float32)
        wm2[:, 2 * g] = (scale * (Wk64[:, gs] @ bq64[gs])).astype(np.float32)
    ybias = (bv64 @ Wfc64 + bfc64).astype(np.float32)

    # exact hi/lo fp8 split of the w-matmul operands
    chi = _q8(wcs)
    chi_f = chi.astype(np.float32)
    chid = _q8(chi_f / 16.0)
    clo = _q8((wcs - chi_f) * 16.0)

    emulr = emuls.astype(np.float32)

    in_maps = []
    for c in range(NCORES):
        b, hg = divmod(c, 2)
        cs = slice(hg * EH, (hg + 1) * EH)
        xT = np.ascontiguousarray(x[b].T)
        x_hi = _q8(xT)
        x_hi_f = x_hi.astype(np.float32)
        in_maps.append({
            "xT8": x_hi,
            "xlo": _q8((xT - x_hi_f) * 16.0),
            "xhid": _q8(xT / 16.0),
            "wa8": _q8(wa[:, cs]),
            "chi": np.ascontiguousarray(chi[:, cs]),
            "chid": np.ascontiguousarray(chid[:, cs]),
            "clo": np.ascontiguousarray(clo[:, cs]),
            "emul": emulr,
            "wones": np.full((P, ST * 2 * (EWW - E)), 2.0 ** KC, np.float32),
            "xTbf": xT.astype(NBF),
            "wm2": np.ascontiguousarray(
                wm2[:, hg * 2 * HPC:(hg + 1) * 2 * HPC]).astype(NBF),
        })
    return struct, has_qk_bias, in_maps, ybias


def kernel(x, attention_mask, Wq, bq, Wk, bk, Wv, bv, Wfc, bfc, _trace=False):
    global LAST_RESULTS
    struct, has_qk_bias, in_maps, ybias = make_core_inputs(
        x, attention_mask, Wq, bq, Wk, bk, Wv, bv, Wfc, bfc)
    key = (struct, has_qk_bias, KA, KC)
    if key not in _nc_cache:
        _nc_cache[key] = _build(struct, has_qk_bias)
    nc = _nc_cache[key]

    from concourse._compat import axon_active
    if axon_active() and not _trace:
        results = _run_pjrt_cached(key, nc, in_maps)
        LAST_RESULTS = None
    else:
        try:
            res = run_bass_kernel_spmd(nc, in_maps, core_ids=list(range(NCORES)),
                                       trace=_trace)
        except ModuleNotFoundError:
            # axon client without NTFF-profiling support: tracing disabled
            import os
            os.environ["BASS_NEVER_TRACE"] = "1"
            res = run_bass_kernel_spmd(nc, in_maps, core_ids=list(range(NCORES)),
                                       trace=False)
        LAST_RESULTS = res
        results = res.results
    out = np.empty((B, S, E), np.float32)
    for b in range(B):
        out[b] = results[2 * b]["y"] + results[2 * b + 1]["y"] + ybias
    return out


_jit_cache = {}


def _run_pjrt_cached(key, nc, in_maps):
    """bass2jax.run_bass_via_pjrt with the sharded jit cached per kernel
    structure, so repeated kernel() calls skip re-tracing (and with it the
    expensive NEFF recompile inside the neuronx_cc hook)."""
    import jax
    from jax.sharding import Mesh, PartitionSpec
    from jax.experimental.shard_map import shard_map
    from concourse import bass2jax
    import concourse.mybir as _mybir

    if key not in _jit_cache:
        bass2jax.install_neuronx_cc_hook()
        in_names, out_names, out_avals, zero_shapes = [], [], [], []
        for alloc in nc.m.functions[0].allocations:
            if not isinstance(alloc, _mybir.MemoryLocationSet):
                continue
            name = alloc.memorylocations[0].name
            if alloc.kind == "ExternalInput":
                if name != "partition_id":
                    in_names.append(name)
            elif alloc.kind == "ExternalOutput":
                shape = tuple(alloc.tensor_shape)
                dtype = _mybir.dt.np(alloc.dtype)
                out_names.append(name)
                out_avals.append(jax.core.ShapedArray(shape, dtype))
                zero_shapes.append((shape, dtype))
        n_params = len(in_names)
        n_outs = len(out_names)
        all_names = in_names + out_names + ["partition_id"]

        def _body(*args):
            operands = list(args)
            operands.append(bass2jax.partition_id_tensor())
            return tuple(bass2jax._bass_exec_p.bind(
                *operands,
                out_avals=tuple(out_avals),
                in_names=tuple(all_names),
                out_names=tuple(out_names),
                lowering_input_output_aliases=(),
                sim_require_finite=True,
                sim_require_nnan=True,
                nc=nc,
            ))

        devices = jax.devices()[:NCORES]
        mesh = Mesh(np.asarray(devices), ("core",))
        sharded = jax.jit(
            shard_map(_body, mesh=mesh,
                      in_specs=(PartitionSpec("core"),) * (n_params + n_outs),
                      out_specs=(PartitionSpec("core"),) * n_outs,
                      check_rep=False),
            donate_argnums=tuple(range(n_params, n_params + n_outs)),
            keep_unused=True,
        )
        _jit_cache[key] = (sharded, in_names, out_names, out_avals, zero_shapes)

    sharded, in_names, out_names, out_avals, zero_shapes = _jit_cache[key]
    concat_in = [
        np.concatenate([np.asarray(m[name]) for m in in_maps], axis=0)
        for name in in_names
    ]

    def _exec():
        concat_zeros = [np.zeros((NCORES * s[0], *s[1:]), d)
                        for s, d in zero_shapes]
        out_arrs = sharded(*concat_in, *concat_zeros)
        return [np.asarray(a) for a in out_arrs]

    try:
        out_arrs = _exec()
    except Exception:
        # transient device/transport flake: drop the failed call's effect
        # tokens (else jax's atexit block_until_ready re-raises even after a
        # successful retry) and retry once with fresh buffers
        try:
            from jax._src import dispatch as _jd
            _jd.runtime_tokens.clear()
        except Exception:
            pass
        out_arrs = _exec()
    return [
        {name: out_arrs[i].reshape(NCORES, *out_avals[i].shape)[c]
         for i, name in enumerate(out_names)}
        for c in range(NCORES)
    ]
